# revision 1
# baseline (speedup 1.0000x reference)
"""KLDiscretLoss joints kernel for TRN2 (8 NeuronCores, Bass/Tile).

Math: for each row (b,j,d) of BINS logits,
  kl_row_sum = sum_bins labels*(log_labels - log_scores)
             = w/St + log(So) - log(St)
  where St = sum(exp(t)), So = sum(exp(o)), w = sum(exp(t)*(t-o)).
(no max-subtraction needed: randn inputs, |x| <~ 6, exp is safe in f32)

Sharding: data-parallel over batch, 32 batches/core -> 1088 rows/core,
tiled as 8x[128,2048] + 1x[64,2048]. Device streams both tensors once
(memory-bound) and emits per-row partial stats; host does the final
per-row combine + batch-mean + sum-over-d + min-over-j in float64.

Schedule notes (from TimelineSim cost model):
- exp on ACT (with fused accum_out row-sum), t-o on Pool/GpSimd,
  mul+reduce on DVE: every engine stays below the ~50us DMA roofline.
- fused tensor_tensor_reduce would save a DVE pass but crashes the NEFF
  on this HW path (NRT_EXEC_UNIT_UNRECOVERABLE) -> mul + reduce.
- the last tiles are bin-halved so the post-last-load dependency chain
  (sub -> mul -> reduce) is half as long; trims ~3us off the tail.
"""

import numpy as np

import concourse.bass as bass
import concourse.tile as tile
from concourse import bacc, mybir
from concourse.bass_utils import run_bass_kernel_spmd

B, J, D, BINS = 256, 17, 2, 2048
NCORES = 8
BS = B // NCORES               # 32 batches per core
ROWS = BS * J * D              # 1088 rows per core
P = 128
NTILES = (ROWS + P - 1) // P   # 9 tiles (8 full + 1 of 64 rows)
SPLIT = {5: 2, 6: 2, 7: 2, 8: 2}  # tail tiles computed in bin-halves
NCOLS = sum(3 * SPLIT.get(i, 1) for i in range(NTILES))
F32 = mybir.dt.float32
Exp = mybir.ActivationFunctionType.Exp
Alu = mybir.AluOpType

_cache = {}


def _build_nc():
    nc = bacc.Bacc(
        "TRN2", target_bir_lowering=False, debug=False, num_devices=NCORES
    )
    o_ap = nc.dram_tensor("o_in", [ROWS, BINS], F32, kind="ExternalInput").ap()
    t_ap = nc.dram_tensor("t_in", [ROWS, BINS], F32, kind="ExternalInput").ap()
    s_ap = nc.dram_tensor("stats", [P, NCOLS], F32, kind="ExternalOutput").ap()

    with tile.TileContext(nc) as tc:
        with (
            tc.tile_pool(name="io", bufs=3) as io,
            tc.tile_pool(name="work", bufs=2) as work,
            tc.tile_pool(name="single", bufs=1) as single,
        ):
            big = single.tile([P, NCOLS], F32)
            nc.vector.memset(big[:], 0.0)
            col = 0
            for i in range(NTILES):
                r0 = i * P
                R = min(P, ROWS - r0)
                nchunk = SPLIT.get(i, 1)
                CS = BINS // nchunk
                for h in range(nchunk):
                    sl = slice(h * CS, (h + 1) * CS)
                    t_t = io.tile([P, BINS], F32, tag="t_t")
                    nc.sync.dma_start(t_t[:R, :CS], t_ap[r0 : r0 + R, sl])
                    o_t = io.tile([P, BINS], F32, tag="o_t")
                    nc.sync.dma_start(o_t[:R, :CS], o_ap[r0 : r0 + R, sl])
                    et = work.tile([P, BINS], F32, tag="et")
                    nc.scalar.activation(
                        et[:R, :CS], t_t[:R, :CS], Exp,
                        accum_out=big[:R, col : col + 1],
                    )
                    eo = work.tile([P, BINS], F32, tag="eo")
                    nc.scalar.activation(
                        eo[:R, :CS], o_t[:R, :CS], Exp,
                        accum_out=big[:R, col + 1 : col + 2],
                    )
                    diff = work.tile([P, BINS], F32, tag="diff")
                    nc.gpsimd.tensor_sub(diff[:R, :CS], t_t[:R, :CS], o_t[:R, :CS])
                    prod = work.tile([P, BINS], F32, tag="prod")
                    nc.vector.tensor_mul(prod[:R, :CS], et[:R, :CS], diff[:R, :CS])
                    nc.vector.tensor_reduce(
                        big[:R, col + 2 : col + 3], prod[:R, :CS],
                        mybir.AxisListType.X, Alu.add,
                    )
                    col += 3
            nc.sync.dma_start(s_ap[:, :], big[:, :])
    nc.compile()
    return nc


def kernel(output, target):
    output = np.ascontiguousarray(output, dtype=np.float32)
    target = np.ascontiguousarray(target, dtype=np.float32)
    assert output.shape == (B, J, D, BINS) and target.shape == (B, J, D, BINS)

    if "nc" not in _cache:
        _cache["nc"] = _build_nc()
    nc = _cache["nc"]

    in_maps = []
    for c in range(NCORES):
        sl = slice(c * BS, (c + 1) * BS)
        in_maps.append(
            {
                "o_in": output[sl].reshape(ROWS, BINS),
                "t_in": target[sl].reshape(ROWS, BINS),
            }
        )

    res = run_bass_kernel_spmd(nc, in_maps, list(range(NCORES)))
    _cache["last_results"] = res

    # host-side decode + final reduction (float64)
    per_row = np.empty((NCORES, ROWS), dtype=np.float64)
    for c in range(NCORES):
        st = res.results[c]["stats"].astype(np.float64)  # [P, NCOLS]
        St = np.zeros((NTILES, P))
        So = np.zeros((NTILES, P))
        w = np.zeros((NTILES, P))
        col = 0
        for i in range(NTILES):
            for _h in range(SPLIT.get(i, 1)):
                St[i] += st[:, col]
                So[i] += st[:, col + 1]
                w[i] += st[:, col + 2]
                col += 3
        St = St.reshape(-1)[:ROWS]
        So = So.reshape(-1)[:ROWS]
        w = w.reshape(-1)[:ROWS]
        per_row[c] = w / St + np.log(So) - np.log(St)

    per_row = per_row.reshape(B, J * D) / BINS          # per_bd, mean over bins
    per_jd = per_row.mean(axis=0)                        # [J*D]
    loss = per_jd.reshape(J, D).sum(axis=1)              # [J]
    return np.float32(loss.min())



# revision 3
# speedup vs baseline: 1.0086x; 1.0086x over previous
"""KLDiscretLoss joints kernel for TRN2 (8 NeuronCores, Bass/Tile).

Math: for each row (b,j,d) of BINS logits,
  kl_row_sum = w/St + log(So) - log(St)
  where St = sum(exp(t)), So = sum(exp(o)), w = sum(exp(t)*(t-o)).
(no max-subtraction needed: randn inputs, |x| <~ 6, exp is safe in f32)

Sharding: data-parallel over batch, 32 batches/core -> 1088 rows/core.
The device streams both tensors once (memory-bound, DMA floor ~49.5us
per core in the cost model) and emits per-row partial stats; host does
the final combine + batch-mean + sum-over-d + min-over-j in float64.

Schedule (v2):
- chunks: 64-row runt tile FIRST, seven full [128,2048] tiles, then the
  last tile as a bin-cascade 1024/512/256/256 so the post-last-load
  dependency chain is short.
- w is computed as  w = sum(et*t) - sum(et*o)  with two fused
  tensor_tensor_reduce passes on DVE (variant "C"); after the final o
  chunk lands only ONE small DVE op remains.  Fallback variant "A"
  (Pool sub + DVE mul/reduce) if TTR misbehaves on HW.
- et is written as bf16 (halves DVE mul cost in variant A; TTR cost is
  element-count-based either way).
- stats leave the device in two DMAs: one bulk DMA covering every chunk
  except the last (issued right after the input loads), plus a tiny
  tail DMA for the last chunk's columns.
"""

import numpy as np

import concourse.bass as bass
import concourse.tile as tile
from concourse import bacc, mybir
from concourse.bass_utils import run_bass_kernel_spmd

B, J, D, BINS = 256, 17, 2, 2048
NCORES = 8
BS = B // NCORES               # 32 batches per core
ROWS = BS * J * D              # 1088 rows per core
P = 128

# (row0, nrows, bin0, nbins) — processing order; LAST chunk tail-optimized
CHUNKS = [
    (1024, 64, 0, 2048),                                  # runt first
    *[(r, 128, 0, 2048) for r in range(0, 896, 128)],     # 7 full tiles
    (896, 128, 0, 1024),                                  # last tile cascade
    (896, 128, 1024, 512),
    (896, 128, 1536, 256),
    (896, 128, 1792, 256),
]
NCHUNKS = len(CHUNKS)

VARIANT = "A"                  # "C": 2x tensor_tensor_reduce; "A": sub+mul+reduce
CPC = 4 if VARIANT == "C" else 3    # stats columns per chunk
NB = (NCHUNKS - 1) * CPC       # bulk stats cols (all chunks but last)
NT = CPC                       # tail stats cols (last chunk)

F32 = mybir.dt.float32
BF16 = mybir.dt.bfloat16
Exp = mybir.ActivationFunctionType.Exp
Alu = mybir.AluOpType

_cache = {}


def _build_nc():
    nc = bacc.Bacc(
        "TRN2", target_bir_lowering=False, debug=False, num_devices=NCORES
    )
    o_ap = nc.dram_tensor("o_in", [ROWS, BINS], F32, kind="ExternalInput").ap()
    t_ap = nc.dram_tensor("t_in", [ROWS, BINS], F32, kind="ExternalInput").ap()
    sb_ap = nc.dram_tensor("stats_bulk", [P, NB], F32, kind="ExternalOutput").ap()
    st_ap = nc.dram_tensor("stats_tail", [P, NT], F32, kind="ExternalOutput").ap()

    with tile.TileContext(nc) as tc:
        with (
            tc.tile_pool(name="io", bufs=4) as io,
            tc.tile_pool(name="work", bufs=3) as work,
            tc.tile_pool(name="single", bufs=1) as single,
        ):
            bulk = single.tile([P, NB], F32)
            tail = single.tile([P, NT], F32)
            eo_scr = single.tile([P, BINS], BF16)   # exp(o) values (unused)
            ttr_scr = single.tile([P, BINS], BF16)  # TTR elementwise out (unused)

            for ci, (r0, R, b0, nb) in enumerate(CHUNKS):
                last = ci == NCHUNKS - 1
                stats = tail if last else bulk
                col = 0 if last else ci * CPC
                rs = slice(r0, r0 + R)
                bs = slice(b0, b0 + nb)

                t_t = io.tile([P, BINS], F32, tag="t_t")
                nc.sync.dma_start(t_t[:R, :nb], t_ap[rs, bs])
                o_t = io.tile([P, BINS], F32, tag="o_t")
                nc.sync.dma_start(o_t[:R, :nb], o_ap[rs, bs])

                et = work.tile([P, BINS], BF16, tag="et")
                nc.scalar.activation(
                    et[:R, :nb], t_t[:R, :nb], Exp,
                    accum_out=stats[:R, col : col + 1],
                )
                nc.scalar.activation(
                    eo_scr[:R, :nb], o_t[:R, :nb], Exp,
                    accum_out=stats[:R, col + 1 : col + 2],
                )
                if VARIANT == "C":
                    # w = sum(et*t) - sum(et*o): two fused mul-reduce passes
                    nc.vector.tensor_tensor_reduce(
                        ttr_scr[:R, :nb], et[:R, :nb], t_t[:R, :nb],
                        1.0, 0.0, Alu.mult, Alu.add,
                        accum_out=stats[:R, col + 2 : col + 3],
                    )
                    nc.vector.tensor_tensor_reduce(
                        ttr_scr[:R, :nb], et[:R, :nb], o_t[:R, :nb],
                        1.0, 0.0, Alu.mult, Alu.add,
                        accum_out=stats[:R, col + 3 : col + 4],
                    )
                else:
                    diff = work.tile([P, BINS], BF16, tag="diff")
                    # last two cascade chunks: keep the chain on DVE
                    sub_eng = nc.vector if ci >= NCHUNKS - 2 else nc.gpsimd
                    sub_eng.tensor_sub(diff[:R, :nb], t_t[:R, :nb], o_t[:R, :nb])
                    prod = work.tile([P, BINS], BF16, tag="prod")
                    nc.vector.tensor_mul(prod[:R, :nb], et[:R, :nb], diff[:R, :nb])
                    nc.vector.tensor_reduce(
                        stats[:R, col + 2 : col + 3], prod[:R, :nb],
                        mybir.AxisListType.X, Alu.add,
                    )
            nc.sync.dma_start(sb_ap[:, :], bulk[:, :])
            nc.sync.dma_start(st_ap[:, :], tail[:, :])
    nc.compile()
    return nc


def kernel(output, target):
    output = np.ascontiguousarray(output, dtype=np.float32)
    target = np.ascontiguousarray(target, dtype=np.float32)
    assert output.shape == (B, J, D, BINS) and target.shape == (B, J, D, BINS)

    if "nc" not in _cache:
        _cache["nc"] = _build_nc()
    nc = _cache["nc"]

    in_maps = []
    for c in range(NCORES):
        sl = slice(c * BS, (c + 1) * BS)
        in_maps.append(
            {
                "o_in": output[sl].reshape(ROWS, BINS),
                "t_in": target[sl].reshape(ROWS, BINS),
            }
        )

    res = run_bass_kernel_spmd(nc, in_maps, list(range(NCORES)))
    _cache["last_results"] = res

    # host-side decode + final reduction (float64)
    per_row = np.empty((NCORES, ROWS), dtype=np.float64)
    for c in range(NCORES):
        sb = res.results[c]["stats_bulk"].astype(np.float64)  # [P, NB]
        st = res.results[c]["stats_tail"].astype(np.float64)  # [P, NT]
        St = np.zeros(ROWS)
        So = np.zeros(ROWS)
        w = np.zeros(ROWS)
        for ci, (r0, R, b0, nb) in enumerate(CHUNKS):
            last = ci == NCHUNKS - 1
            s = st if last else sb
            col = 0 if last else ci * CPC
            rs = slice(r0, r0 + R)
            St[rs] += s[:R, col]
            So[rs] += s[:R, col + 1]
            if VARIANT == "C":
                w[rs] += s[:R, col + 2] - s[:R, col + 3]
            else:
                w[rs] += s[:R, col + 2]
        per_row[c] = w / St + np.log(So) - np.log(St)

    per_row = per_row.reshape(B, J * D) / BINS            # per_bd, mean over bins
    per_jd = per_row.mean(axis=0)                         # [J*D]
    loss = per_jd.reshape(J, D).sum(axis=1)               # [J]
    return np.float32(loss.min())


# revision 4
# speedup vs baseline: 1.0249x; 1.0162x over previous
"""KLDiscretLoss joints kernel for TRN2 (8 NeuronCores, Bass/Tile).

Math: for each row (b,j,d) of BINS logits,
  kl_row_sum = w/St + log(So) - log(St)
  where St = sum(exp(t)), So = sum(exp(o)), w = sum(exp(t)*(t-o)).
(no max-subtraction needed: randn inputs, |x| <~ 6, exp is safe in f32)

Sharding: data-parallel over batch, 32 batches/core -> 1088 rows/core.
The device streams both tensors once (memory-bound, DMA floor ~49.5us
per core in the cost model) and emits per-row partial stats; host does
the final combine + batch-mean + sum-over-d + min-over-j in float64.

Schedule (v4):
- w per chunk = one sub (t-o -> bf16 diff) + ONE fused DVE op:
  scalar_tensor_tensor(out=(et*1.0)*diff, accum_out=row-sum) -- the
  accumulator replaces the separate tensor_reduce pass entirely.
  (tensor_tensor_reduce would fuse even further but NEFF-crashes on HW.)
- chunk order: 64-row runt tile FIRST, seven full [128,2048] tiles,
  then the last tile as a bin-cascade 1024/512/256/256 so the
  post-last-load dependency chain is a few hundred ns of DVE work.
- subs of big chunks go to Pool/GpSimd (keeps DVE slack); cascade subs
  stay on DVE so the tail chain has no cross-engine hops.
- et is bf16 (smaller SBUF, faster mul path); accumulators stay f32.
- stats leave the device in two DMAs: a bulk DMA covering the early
  chunks (issued right after the input loads) and a tiny tail DMA for
  the cascade columns.
"""

import numpy as np

import concourse.bass as bass
import concourse.tile as tile
from concourse import bacc, mybir
from concourse.bass_utils import run_bass_kernel_spmd

B, J, D, BINS = 256, 17, 2, 2048
NCORES = 8
BS = B // NCORES               # 32 batches per core
ROWS = BS * J * D              # 1088 rows per core
P = 128

# (row0, nrows, bin0, nbins) in processing order; last chunks tail-optimized
CHUNKS = [
    (1024, 64, 0, 2048),                                  # runt first
    *[(r, 128, 0, 2048) for r in range(0, 896, 128)],     # 7 full tiles
    (896, 128, 0, 1024),                                  # last-tile cascade
    (896, 128, 1024, 512),
    (896, 128, 1536, 256),
    (896, 128, 1792, 256),
]
NCHUNKS = len(CHUNKS)
NCASCADE = 4                   # trailing chunks whose stats go in the tail DMA
NBULK = NCHUNKS - NCASCADE

CPC = 3                        # stats columns per chunk: St, So, w
NB = NBULK * CPC
NT = NCASCADE * CPC

F32 = mybir.dt.float32
BF16 = mybir.dt.bfloat16
Exp = mybir.ActivationFunctionType.Exp
Alu = mybir.AluOpType

_cache = {}


def _build_nc():
    nc = bacc.Bacc(
        "TRN2", target_bir_lowering=False, debug=False, num_devices=NCORES
    )
    o_ap = nc.dram_tensor("o_in", [ROWS, BINS], F32, kind="ExternalInput").ap()
    t_ap = nc.dram_tensor("t_in", [ROWS, BINS], F32, kind="ExternalInput").ap()
    sb_ap = nc.dram_tensor("stats_bulk", [P, NB], F32, kind="ExternalOutput").ap()
    st_ap = nc.dram_tensor("stats_tail", [P, NT], F32, kind="ExternalOutput").ap()

    with tile.TileContext(nc) as tc:
        with (
            tc.tile_pool(name="io", bufs=6) as io,
            tc.tile_pool(name="work", bufs=3) as work,
            tc.tile_pool(name="single", bufs=1) as single,
        ):
            bulk = single.tile([P, NB], F32)
            tail = single.tile([P, NT], F32)
            eo_scr = single.tile([P, BINS], BF16)   # exp(o) values (unused)
            stt_scr = single.tile([P, BINS], BF16)  # STT elementwise out (unused)

            for ci, (r0, R, b0, nb) in enumerate(CHUNKS):
                casc = ci >= NBULK
                stats = tail if casc else bulk
                col = (ci - NBULK) * CPC if casc else ci * CPC
                rs = slice(r0, r0 + R)
                bsl = slice(b0, b0 + nb)

                t_t = io.tile([P, BINS], F32, tag="t_t")
                nc.sync.dma_start(t_t[:R, :nb], t_ap[rs, bsl])
                o_t = io.tile([P, BINS], F32, tag="o_t")
                nc.sync.dma_start(o_t[:R, :nb], o_ap[rs, bsl])

                et = work.tile([P, BINS], BF16, tag="et")
                nc.scalar.activation(
                    et[:R, :nb], t_t[:R, :nb], Exp,
                    accum_out=stats[:R, col : col + 1],
                )
                nc.scalar.activation(
                    eo_scr[:R, :nb], o_t[:R, :nb], Exp,
                    accum_out=stats[:R, col + 1 : col + 2],
                )
                diff = work.tile([P, BINS], BF16, tag="diff")
                sub_eng = nc.vector if casc else nc.gpsimd
                sub_eng.tensor_sub(diff[:R, :nb], t_t[:R, :nb], o_t[:R, :nb])
                nc.vector.scalar_tensor_tensor(
                    stt_scr[:R, :nb], et[:R, :nb], 1.0, diff[:R, :nb],
                    Alu.mult, Alu.mult,
                    accum_out=stats[:R, col + 2 : col + 3],
                )
            nc.sync.dma_start(sb_ap[:, :], bulk[:, :])
            nc.sync.dma_start(st_ap[:, :], tail[:, :])
    nc.compile()
    return nc


def kernel(output, target):
    output = np.ascontiguousarray(output, dtype=np.float32)
    target = np.ascontiguousarray(target, dtype=np.float32)
    assert output.shape == (B, J, D, BINS) and target.shape == (B, J, D, BINS)

    if "nc" not in _cache:
        _cache["nc"] = _build_nc()
    nc = _cache["nc"]

    in_maps = []
    for c in range(NCORES):
        sl = slice(c * BS, (c + 1) * BS)
        in_maps.append(
            {
                "o_in": output[sl].reshape(ROWS, BINS),
                "t_in": target[sl].reshape(ROWS, BINS),
            }
        )

    res = run_bass_kernel_spmd(nc, in_maps, list(range(NCORES)))
    _cache["last_results"] = res

    # host-side decode + final reduction (float64)
    per_row = np.empty((NCORES, ROWS), dtype=np.float64)
    for c in range(NCORES):
        sb = res.results[c]["stats_bulk"].astype(np.float64)  # [P, NB]
        st = res.results[c]["stats_tail"].astype(np.float64)  # [P, NT]
        St = np.zeros(ROWS)
        So = np.zeros(ROWS)
        w = np.zeros(ROWS)
        for ci, (r0, R, b0, nb) in enumerate(CHUNKS):
            casc = ci >= NBULK
            s = st if casc else sb
            col = (ci - NBULK) * CPC if casc else ci * CPC
            rs = slice(r0, r0 + R)
            St[rs] += s[:R, col]
            So[rs] += s[:R, col + 1]
            w[rs] += s[:R, col + 2]
        per_row[c] = w / St + np.log(So) - np.log(St)

    per_row = per_row.reshape(B, J * D) / BINS            # per_bd, mean over bins
    per_jd = per_row.mean(axis=0)                         # [J*D]
    loss = per_jd.reshape(J, D).sum(axis=1)               # [J]
    return np.float32(loss.min())


# revision 5
# speedup vs baseline: 1.0526x; 1.0270x over previous
"""KLDiscretLoss joints kernel for TRN2 (8 NeuronCores, Bass/Tile).

Math: for each row (b,j,d) of BINS logits,
  kl_row_sum = w/St + log(So) - log(St)
  where St = sum(exp(t)), So = sum(exp(o)), w = sum(exp(t)*(t-o)).
(no max-subtraction needed: randn inputs, |x| <~ 6, exp is safe in f32)

Sharding: data-parallel over batch, 32 batches/core -> 1088 rows/core.
The device streams both tensors once (memory-bound, DMA floor ~49.5us
per core in the cost model) and emits per-row partial stats; host does
the final combine + batch-mean + sum-over-d + min-over-j in float64.

Schedule (v5):
- w per chunk = sub (t-o -> bf16) + ONE fused DVE scalar_tensor_tensor
  whose accum_out is the row-sum (replaces mul+tensor_reduce; the more
  aggressive tensor_tensor_reduce NEFF-crashes on HW).
- chunk order: 64-row runt tile FIRST, six full [128,2048] tiles
  (Pool subs / DVE STT), then T6 in two 1024-bin pieces and T7 in a
  1024/512/256/256 bin-cascade so the post-last-load chain is short.
- T6/T7 stream into persistent SBUF tiles via piece-DMAs; subtile deps
  let ACT use coarser chunks (amortizing its ~430ns/op overhead) than
  the DVE w-chain: T7 exp_t per load piece, exp_o as 1024/512/512.
- T7 cascade subs alternate DVE (c1,c3) / Pool (c2,c4) so the two
  engines drain the tail in parallel.
- stats leave in two DMAs: bulk (runt..T6) right after the loads, and
  a tiny tail DMA with T7's 11 columns.
"""

import numpy as np

import concourse.bass as bass
import concourse.tile as tile
from concourse import bacc, mybir
from concourse.bass_utils import run_bass_kernel_spmd

B, J, D, BINS = 256, 17, 2, 2048
NCORES = 8
BS = B // NCORES               # 32 batches per core
ROWS = BS * J * D              # 1088 rows per core
P = 128

# standard full-bin chunks: (row0, nrows) — runt first, then T0..T5
STD = [(1024, 64)] + [(r, 128) for r in range(0, 768, 128)]
T6_R0, T7_R0 = 768, 896
T6_PIECES = [(0, 1024), (1024, 1024)]            # load/compute pieces
T7_LOAD = [(0, 1024), (1024, 512), (1536, 256), (1792, 256)]
T7_SO = [(0, 1024), (1024, 512), (1536, 512)]    # exp_o granularity
# bulk stats cols: STD (3 each) + T6 pieces (3 each)
NB = 3 * len(STD) + 3 * len(T6_PIECES)           # 27
# tail stats cols: T7 St per load piece, So per SO piece, w per load piece
NT = len(T7_LOAD) + len(T7_SO) + len(T7_LOAD)    # 11

F32 = mybir.dt.float32
BF16 = mybir.dt.bfloat16
Exp = mybir.ActivationFunctionType.Exp
Alu = mybir.AluOpType

_cache = {}


def _build_nc():
    nc = bacc.Bacc(
        "TRN2", target_bir_lowering=False, debug=False, num_devices=NCORES
    )
    o_ap = nc.dram_tensor("o_in", [ROWS, BINS], F32, kind="ExternalInput").ap()
    t_ap = nc.dram_tensor("t_in", [ROWS, BINS], F32, kind="ExternalInput").ap()
    sb_ap = nc.dram_tensor("stats_bulk", [P, NB], F32, kind="ExternalOutput").ap()
    st_ap = nc.dram_tensor("stats_tail", [P, NT], F32, kind="ExternalOutput").ap()

    with tile.TileContext(nc) as tc:
        with (
            tc.tile_pool(name="io", bufs=4) as io,
            tc.tile_pool(name="work", bufs=3) as work,
            tc.tile_pool(name="single", bufs=1) as single,
        ):
            bulk = single.tile([P, NB], F32)
            tail = single.tile([P, NT], F32)
            eo_scr = single.tile([P, BINS], BF16)   # exp(o) values (unused)
            stt_scr = single.tile([P, BINS], BF16)  # STT elementwise out (unused)

            def wchain(stats, col, et_sl, t_sl, o_sl, diff_sl, R, sub_eng):
                sub_eng.tensor_sub(diff_sl, t_sl, o_sl)
                nc.vector.scalar_tensor_tensor(
                    stt_scr[:R, : t_sl.shape[1]], et_sl, 1.0, diff_sl,
                    Alu.mult, Alu.mult,
                    accum_out=stats[:R, col : col + 1],
                )

            # --- standard chunks: runt, T0..T5 ---
            for ci, (r0, R) in enumerate(STD):
                col = 3 * ci
                rs = slice(r0, r0 + R)
                t_t = io.tile([P, BINS], F32, tag="t_t")
                nc.sync.dma_start(t_t[:R, :], t_ap[rs, :])
                o_t = io.tile([P, BINS], F32, tag="o_t")
                nc.sync.dma_start(o_t[:R, :], o_ap[rs, :])
                et = work.tile([P, BINS], BF16, tag="et")
                nc.scalar.activation(
                    et[:R, :], t_t[:R, :], Exp,
                    accum_out=bulk[:R, col : col + 1],
                )
                nc.scalar.activation(
                    eo_scr[:R, :], o_t[:R, :], Exp,
                    accum_out=bulk[:R, col + 1 : col + 2],
                )
                diff = work.tile([P, BINS], BF16, tag="diff")
                wchain(bulk, col + 2, et[:R, :], t_t[:R, :], o_t[:R, :],
                       diff[:R, :], R, nc.gpsimd)

            # --- T6: two 1024-bin pieces through persistent tiles ---
            t6t = single.tile([P, BINS], F32)
            t6o = single.tile([P, BINS], F32)
            et6 = single.tile([P, BINS], BF16)
            df6 = single.tile([P, BINS], BF16)
            rs6 = slice(T6_R0, T6_R0 + P)
            for pi, (b0, nb) in enumerate(T6_PIECES):
                col = 3 * len(STD) + 3 * pi
                bsl = slice(b0, b0 + nb)
                nc.sync.dma_start(t6t[:, bsl], t_ap[rs6, bsl])
                nc.sync.dma_start(t6o[:, bsl], o_ap[rs6, bsl])
                nc.scalar.activation(
                    et6[:, bsl], t6t[:, bsl], Exp,
                    accum_out=bulk[:, col : col + 1],
                )
                nc.scalar.activation(
                    eo_scr[:, bsl], t6o[:, bsl], Exp,
                    accum_out=bulk[:, col + 1 : col + 2],
                )
                wchain(bulk, col + 2, et6[:, bsl], t6t[:, bsl], t6o[:, bsl],
                       df6[:, bsl], P, nc.vector)

            # --- T7: cascade through persistent tiles ---
            t7t = single.tile([P, BINS], F32)
            t7o = single.tile([P, BINS], F32)
            et7 = single.tile([P, BINS], BF16)
            df7 = single.tile([P, BINS], BF16)
            rs7 = slice(T7_R0, T7_R0 + P)
            # loads in pair order (t then o per piece)
            for b0, nb in T7_LOAD:
                bsl = slice(b0, b0 + nb)
                nc.sync.dma_start(t7t[:, bsl], t_ap[rs7, bsl])
                nc.sync.dma_start(t7o[:, bsl], o_ap[rs7, bsl])

            def t7_expt(pi):
                b0, nb = T7_LOAD[pi]
                bsl = slice(b0, b0 + nb)
                nc.scalar.activation(
                    et7[:, bsl], t7t[:, bsl], Exp,
                    accum_out=tail[:, pi : pi + 1],
                )

            def t7_expo(si):
                b0, nb = T7_SO[si]
                c = len(T7_LOAD) + si
                bsl = slice(b0, b0 + nb)
                nc.scalar.activation(
                    eo_scr[:, bsl], t7o[:, bsl], Exp,
                    accum_out=tail[:, c : c + 1],
                )

            def t7_sub(wi, eng):
                b0, nb = T7_LOAD[wi]
                bsl = slice(b0, b0 + nb)
                eng.tensor_sub(df7[:, bsl], t7t[:, bsl], t7o[:, bsl])

            def t7_stt(wi):
                b0, nb = T7_LOAD[wi]
                c = len(T7_LOAD) + len(T7_SO) + wi
                bsl = slice(b0, b0 + nb)
                nc.vector.scalar_tensor_tensor(
                    stt_scr[:, bsl], et7[:, bsl], 1.0, df7[:, bsl],
                    Alu.mult, Alu.mult,
                    accum_out=tail[:, c : c + 1],
                )

            # ACT queue order: each op as early as its data allows
            t7_expt(0); t7_expo(0); t7_expt(1); t7_expo(1)
            t7_expt(2); t7_expt(3); t7_expo(2)
            # DVE queue: c1 sub+STT, c3 sub, then STTs as diffs arrive
            t7_sub(0, nc.vector); t7_stt(0)
            t7_sub(2, nc.vector)
            # Pool drains c2/c4 subs in parallel with DVE
            t7_sub(1, nc.gpsimd); t7_sub(3, nc.gpsimd)
            t7_stt(1); t7_stt(2); t7_stt(3)

            nc.sync.dma_start(sb_ap[:, :], bulk[:, :])
            nc.sync.dma_start(st_ap[:, :], tail[:, :])
    nc.compile()
    return nc


def kernel(output, target):
    output = np.ascontiguousarray(output, dtype=np.float32)
    target = np.ascontiguousarray(target, dtype=np.float32)
    assert output.shape == (B, J, D, BINS) and target.shape == (B, J, D, BINS)

    if "nc" not in _cache:
        _cache["nc"] = _build_nc()
    nc = _cache["nc"]

    in_maps = []
    for c in range(NCORES):
        sl = slice(c * BS, (c + 1) * BS)
        in_maps.append(
            {
                "o_in": output[sl].reshape(ROWS, BINS),
                "t_in": target[sl].reshape(ROWS, BINS),
            }
        )

    res = run_bass_kernel_spmd(nc, in_maps, list(range(NCORES)))
    _cache["last_results"] = res

    # host-side decode + final reduction (float64)
    per_row = np.empty((NCORES, ROWS), dtype=np.float64)
    for c in range(NCORES):
        sb = res.results[c]["stats_bulk"].astype(np.float64)  # [P, NB]
        st = res.results[c]["stats_tail"].astype(np.float64)  # [P, NT]
        St = np.zeros(ROWS)
        So = np.zeros(ROWS)
        w = np.zeros(ROWS)
        for ci, (r0, R) in enumerate(STD):
            rs = slice(r0, r0 + R)
            St[rs] += sb[:R, 3 * ci]
            So[rs] += sb[:R, 3 * ci + 1]
            w[rs] += sb[:R, 3 * ci + 2]
        rs = slice(T6_R0, T6_R0 + P)
        for pi in range(len(T6_PIECES)):
            col = 3 * len(STD) + 3 * pi
            St[rs] += sb[:, col]
            So[rs] += sb[:, col + 1]
            w[rs] += sb[:, col + 2]
        rs = slice(T7_R0, T7_R0 + P)
        for pi in range(len(T7_LOAD)):
            St[rs] += st[:, pi]
            w[rs] += st[:, len(T7_LOAD) + len(T7_SO) + pi]
        for si in range(len(T7_SO)):
            So[rs] += st[:, len(T7_LOAD) + si]
        per_row[c] = w / St + np.log(So) - np.log(St)

    per_row = per_row.reshape(B, J * D) / BINS            # per_bd, mean over bins
    per_jd = per_row.mean(axis=0)                         # [J*D]
    loss = per_jd.reshape(J, D).sum(axis=1)               # [J]
    return np.float32(loss.min())


# revision 6
# speedup vs baseline: 1.0922x; 1.0376x over previous
"""KLDiscretLoss joints kernel for TRN2 (8 NeuronCores, Bass/Tile).

Math: for each row (b,j,d) of BINS logits,
  kl_row_sum = w/St + log(So) - log(St)
  where St = sum(exp(t)), So = sum(exp(o)), w = sum(exp(t)*(t-o)).
(no max-subtraction needed: randn inputs, |x| <~ 6, exp is safe in f32)

Sharding: data-parallel over batch, 32 batches/core -> 1088 rows/core.
The device streams both tensors once (memory-bound, DMA floor ~49.5us
per core in the cost model) and emits per-row partial stats; host does
the final combine + batch-mean + sum-over-d + min-over-j in float64.

Schedule (v6):
- w per chunk = sub (t-o -> bf16) + ONE fused DVE scalar_tensor_tensor
  whose accum_out is the row-sum (replaces mul+tensor_reduce; the more
  aggressive tensor_tensor_reduce NEFF-crashes on HW).
- chunk order: 64-row runt FIRST, then T0..T5 full tiles, then T6 in
  two 1024-bin pieces, then T7 as the tail cascade.
- engine balance at the end (engines run their queues in order, so the
  trailing queue of each engine must be short): T0..T4 subs on Pool,
  T5 + T6c2 subs in DVE's idle window, T6c1 on Pool; T7 subs c1/c2 on
  DVE, c3/c4 on Pool; all STTs on DVE.
- T7 streams into persistent tiles as t[0:1024], o[0:1024],
  t[1024:2048], o[1024:1536], o[1536:1792], o[1792:2048]; subtile deps
  let ACT run just 4 coarse 1024-bin exps (its ~430ns/op overhead
  would otherwise pile up at the end) while the DVE w-chain uses
  1024/512/256/256 chunks whose last link is a few-hundred-ns op.
- stats leave in two DMAs: bulk (runt..T6, 27 cols) right after the
  loads, and a tiny tail DMA with T7's 8 columns.
"""

import numpy as np

import concourse.bass as bass
import concourse.tile as tile
from concourse import bacc, mybir
from concourse.bass_utils import run_bass_kernel_spmd

B, J, D, BINS = 256, 17, 2, 2048
NCORES = 8
BS = B // NCORES               # 32 batches per core
ROWS = BS * J * D              # 1088 rows per core
P = 128

# standard full-bin chunks: (row0, nrows) — runt first, then T0..T5
STD = [(1024, 64)] + [(r, 128) for r in range(0, 768, 128)]
T6_R0, T7_R0 = 768, 896
T6_PIECES = [(0, 1024), (1024, 1024)]
T7_ACT = [(0, 1024), (1024, 1024)]               # exp_t / exp_o granularity
T7_W = [(0, 1024), (1024, 512), (1536, 256), (1792, 256)]
NB = 3 * len(STD) + 3 * len(T6_PIECES)           # 27 bulk cols
NT = 2 * len(T7_ACT) + len(T7_W)                 # 8 tail cols: St x2, So x2, w x4

F32 = mybir.dt.float32
BF16 = mybir.dt.bfloat16
Exp = mybir.ActivationFunctionType.Exp
Alu = mybir.AluOpType

_cache = {}


def _build_nc():
    nc = bacc.Bacc(
        "TRN2", target_bir_lowering=False, debug=False, num_devices=NCORES
    )
    o_ap = nc.dram_tensor("o_in", [ROWS, BINS], F32, kind="ExternalInput").ap()
    t_ap = nc.dram_tensor("t_in", [ROWS, BINS], F32, kind="ExternalInput").ap()
    sb_ap = nc.dram_tensor("stats_bulk", [P, NB], F32, kind="ExternalOutput").ap()
    st_ap = nc.dram_tensor("stats_tail", [P, NT], F32, kind="ExternalOutput").ap()

    with tile.TileContext(nc) as tc:
        with (
            tc.tile_pool(name="io", bufs=4) as io,
            tc.tile_pool(name="work", bufs=3) as work,
            tc.tile_pool(name="single", bufs=1) as single,
        ):
            bulk = single.tile([P, NB], F32)
            tail = single.tile([P, NT], F32)
            eo_scr = single.tile([P, BINS], BF16)   # exp(o) values (unused)
            stt_scr = single.tile([P, BINS], BF16)  # STT elementwise out (unused)

            def stt(stats, col, et_sl, diff_sl, scr_sl):
                nc.vector.scalar_tensor_tensor(
                    scr_sl, et_sl, 1.0, diff_sl, Alu.mult, Alu.mult,
                    accum_out=stats[:, col : col + 1],
                )

            # --- standard chunks: runt, T0..T5 ---
            for ci, (r0, R) in enumerate(STD):
                col = 3 * ci
                rs = slice(r0, r0 + R)
                t_t = io.tile([P, BINS], F32, tag="t_t")
                nc.sync.dma_start(t_t[:R, :], t_ap[rs, :])
                o_t = io.tile([P, BINS], F32, tag="o_t")
                nc.sync.dma_start(o_t[:R, :], o_ap[rs, :])
                et = work.tile([P, BINS], BF16, tag="et")
                nc.scalar.activation(
                    et[:R, :], t_t[:R, :], Exp,
                    accum_out=bulk[:R, col : col + 1],
                )
                nc.scalar.activation(
                    eo_scr[:R, :], o_t[:R, :], Exp,
                    accum_out=bulk[:R, col + 1 : col + 2],
                )
                diff = work.tile([P, BINS], BF16, tag="diff")
                # T5 (last std chunk) subs on DVE to shorten Pool's ladder
                sub_eng = nc.vector if ci == len(STD) - 1 else nc.gpsimd
                sub_eng.tensor_sub(diff[:R, :], t_t[:R, :], o_t[:R, :])
                nc.vector.scalar_tensor_tensor(
                    stt_scr[:R, :], et[:R, :], 1.0, diff[:R, :],
                    Alu.mult, Alu.mult,
                    accum_out=bulk[:R, col + 2 : col + 3],
                )

            # --- T6: two 1024-bin pieces through persistent tiles ---
            t6t = single.tile([P, BINS], F32)
            t6o = single.tile([P, BINS], F32)
            et6 = single.tile([P, BINS], BF16)
            df6 = single.tile([P, BINS], BF16)
            rs6 = slice(T6_R0, T6_R0 + P)
            for pi, (b0, nb) in enumerate(T6_PIECES):
                col = 3 * len(STD) + 3 * pi
                bsl = slice(b0, b0 + nb)
                nc.sync.dma_start(t6t[:, bsl], t_ap[rs6, bsl])
                nc.sync.dma_start(t6o[:, bsl], o_ap[rs6, bsl])
                nc.scalar.activation(
                    et6[:, bsl], t6t[:, bsl], Exp,
                    accum_out=bulk[:, col : col + 1],
                )
                nc.scalar.activation(
                    eo_scr[:, bsl], t6o[:, bsl], Exp,
                    accum_out=bulk[:, col + 1 : col + 2],
                )
                sub_eng = nc.gpsimd if pi == 0 else nc.vector
                sub_eng.tensor_sub(df6[:, bsl], t6t[:, bsl], t6o[:, bsl])
                stt(bulk, col + 2, et6[:, bsl], df6[:, bsl], stt_scr[:, bsl])

            # --- T7: tail cascade through persistent tiles ---
            t7t = single.tile([P, BINS], F32)
            t7o = single.tile([P, BINS], F32)
            et7 = single.tile([P, BINS], BF16)
            df7 = single.tile([P, BINS], BF16)
            rs7 = slice(T7_R0, T7_R0 + P)
            # loads: t half 1, o half 1, t half 2, then o in 512/256/256 pieces
            nc.sync.dma_start(t7t[:, 0:1024], t_ap[rs7, 0:1024])
            nc.sync.dma_start(t7o[:, 0:1024], o_ap[rs7, 0:1024])
            nc.sync.dma_start(t7t[:, 1024:2048], t_ap[rs7, 1024:2048])
            nc.sync.dma_start(t7o[:, 1024:1536], o_ap[rs7, 1024:1536])
            nc.sync.dma_start(t7o[:, 1536:1792], o_ap[rs7, 1536:1792])
            nc.sync.dma_start(t7o[:, 1792:2048], o_ap[rs7, 1792:2048])

            # ACT: 4 coarse ops, emitted in data-arrival order
            for ai, (b0, nb) in enumerate(T7_ACT):
                bsl = slice(b0, b0 + nb)
                nc.scalar.activation(
                    et7[:, bsl], t7t[:, bsl], Exp,
                    accum_out=tail[:, ai : ai + 1],
                )
                c = len(T7_ACT) + ai
                nc.scalar.activation(
                    eo_scr[:, bsl], t7o[:, bsl], Exp,
                    accum_out=tail[:, c : c + 1],
                )

            # w-chain: subs c1/c2 on DVE, c3/c4 on Pool; STTs on DVE
            w0 = 2 * len(T7_ACT)
            sls = [slice(b0, b0 + nb) for b0, nb in T7_W]
            nc.vector.tensor_sub(df7[:, sls[0]], t7t[:, sls[0]], t7o[:, sls[0]])
            stt(tail, w0 + 0, et7[:, sls[0]], df7[:, sls[0]], stt_scr[:, sls[0]])
            nc.vector.tensor_sub(df7[:, sls[1]], t7t[:, sls[1]], t7o[:, sls[1]])
            nc.gpsimd.tensor_sub(df7[:, sls[2]], t7t[:, sls[2]], t7o[:, sls[2]])
            nc.gpsimd.tensor_sub(df7[:, sls[3]], t7t[:, sls[3]], t7o[:, sls[3]])
            stt(tail, w0 + 1, et7[:, sls[1]], df7[:, sls[1]], stt_scr[:, sls[1]])
            stt(tail, w0 + 2, et7[:, sls[2]], df7[:, sls[2]], stt_scr[:, sls[2]])
            stt(tail, w0 + 3, et7[:, sls[3]], df7[:, sls[3]], stt_scr[:, sls[3]])

            nc.sync.dma_start(sb_ap[:, :], bulk[:, :])
            nc.sync.dma_start(st_ap[:, :], tail[:, :])
    nc.compile()
    return nc


def kernel(output, target):
    output = np.ascontiguousarray(output, dtype=np.float32)
    target = np.ascontiguousarray(target, dtype=np.float32)
    assert output.shape == (B, J, D, BINS) and target.shape == (B, J, D, BINS)

    if "nc" not in _cache:
        _cache["nc"] = _build_nc()
    nc = _cache["nc"]

    in_maps = []
    for c in range(NCORES):
        sl = slice(c * BS, (c + 1) * BS)
        in_maps.append(
            {
                "o_in": output[sl].reshape(ROWS, BINS),
                "t_in": target[sl].reshape(ROWS, BINS),
            }
        )

    res = run_bass_kernel_spmd(nc, in_maps, list(range(NCORES)))
    _cache["last_results"] = res

    # host-side decode + final reduction (float64)
    per_row = np.empty((NCORES, ROWS), dtype=np.float64)
    for c in range(NCORES):
        sb = res.results[c]["stats_bulk"].astype(np.float64)  # [P, NB]
        st = res.results[c]["stats_tail"].astype(np.float64)  # [P, NT]
        St = np.zeros(ROWS)
        So = np.zeros(ROWS)
        w = np.zeros(ROWS)
        for ci, (r0, R) in enumerate(STD):
            rs = slice(r0, r0 + R)
            St[rs] += sb[:R, 3 * ci]
            So[rs] += sb[:R, 3 * ci + 1]
            w[rs] += sb[:R, 3 * ci + 2]
        rs = slice(T6_R0, T6_R0 + P)
        for pi in range(len(T6_PIECES)):
            col = 3 * len(STD) + 3 * pi
            St[rs] += sb[:, col]
            So[rs] += sb[:, col + 1]
            w[rs] += sb[:, col + 2]
        rs = slice(T7_R0, T7_R0 + P)
        for ai in range(len(T7_ACT)):
            St[rs] += st[:, ai]
            So[rs] += st[:, len(T7_ACT) + ai]
        for wi in range(len(T7_W)):
            w[rs] += st[:, 2 * len(T7_ACT) + wi]
        per_row[c] = w / St + np.log(So) - np.log(St)

    per_row = per_row.reshape(B, J * D) / BINS            # per_bd, mean over bins
    per_jd = per_row.mean(axis=0)                         # [J*D]
    loss = per_jd.reshape(J, D).sum(axis=1)               # [J]
    return np.float32(loss.min())
